# revision 76
# baseline (speedup 1.0000x reference)
"""Multi-head attention (RoPE, non-causal) on 8 Trainium2 cores.

Sharding: DP=2 over batch x TP=4 over heads (4 heads per core).
Each core computes, for its (batch, head-group):
    kT   = RoPE(x @ wk.T).T   [per head: 128 x S, head-dim on partitions]
    v    = x @ wv.T           [S x 512, tokens on partitions]
  then per 512-token q-block:
    qT     = RoPE(x @ wq.T).T            [bf16]
    expT   = exp(scale * k.T q)          [transposed scores, bf16]
    denom  = ones^T (wide-tile add tree of expT on DVE)  [1 PE matmul]
    avT    = (v.T @ expT) * recip(denom)
    yT_partial = woT.T-slice @ avT        [2048 x 512]
Host sums the 4 head-group partials per batch and transposes.

All matmul operands are bf16 (same PE column rate as fp32r, half the
SBUF/DMA traffic); accumulation stays fp32 in PSUM; y/cos/sin are bf16
(fp8 measured 3.7% max-rel error in simulation — over the 2e-2 budget).

Schedule: x is loaded ONCE and stays SBUF-resident (64KB/partition);
phase 1 interleaves k-proj/v-proj across token blocks (k1,k2,v1,k3,v2,
v3,k0,q0,v0) so the kernel head only needs x(tb1)+wk d-major slice
+cos/sin before full-rate matmuls; ~4us of dummy matmuls on memset data
warm the HAM clock gate during the initial DMA wait. wq/wk live d-major
in DRAM (one contiguous 512KB DMA per head). In phase 2 every
out-projection is DISSOLVED into the following attention block (chains
spread over heads, next-block q-projection groups after each head) so
the exp stream never sees an idle window and the PE never waits on exp
at block boundaries; each head's last two AV matmuls + denominator ride
in the next head's stream, the final head-3 tail under the last
out-projection's opened chains. PSUM pool order (psav,psQD,pss) pins
scores to banks 4-7 so attn(0) never WAR-waits on phase-1 psum drains.
The reciprocal row is broadcast on gpsimd; all y copies ride the DVE
(scalar stays pure-exp) and y DMAs the sync queue (an active gpsimd DMA
ring at kernel end costs ~8us of serial drain), with the final block
split sync/scalar.
"""

import sys
import types
import numpy as np

HIDDEN = 2048
NUM_HEADS = 16
HEAD_DIM = 128
ROPE_BASE = 10000.0
BATCH = 2
SEQ = 2048

N_CORES = 8
DP = 2            # batch shards
TP = 4            # head-group shards
HPC = NUM_HEADS // TP          # heads per core = 4
DPC = HPC * HEAD_DIM           # head dims per core = 512
P = 128                        # partitions
TB = 512                       # token block (matmul free dim)

FP8_AV = False                 # fp8 AV numerator costs ~3% error: too much
AVMUL_GPSIMD = False           # scalar-engine staging blocks exp: keep on DVE


def _ensure_axon_hooks():
    """bass_utils' trace path imports antenv.axon_hooks, which the container's
    antenv stub lacks. Provide it, backed by the ctypes NTFF hook."""
    import antenv

    if hasattr(antenv, "axon_hooks"):
        return
    try:
        from trn_agent_boot.trn_boot import _ntff_profile_via_ctypes

        hook = _ntff_profile_via_ctypes("/opt/axon/libaxon_pjrt.so")
    except Exception:
        hook = None
    m = types.ModuleType("antenv.axon_hooks")
    m.get_axon_ntff_profile_hook = lambda: hook
    sys.modules["antenv.axon_hooks"] = m
    antenv.axon_hooks = m


def build(seq=SEQ):
    """Build the per-core Bass program (SPMD: same program, per-core data)."""
    import concourse.tile as tile
    from concourse import bacc, mybir

    F32 = mybir.dt.float32
    F32R = mybir.dt.float32r
    BF16 = mybir.dt.bfloat16
    FP8 = mybir.dt.float8e4
    EX_DT = FP8 if FP8_AV else BF16
    EXP_BIAS = -2.0 if FP8_AV else 0.0
    DR = mybir.MatmulPerfMode.DoubleRow
    Exp = mybir.ActivationFunctionType.Exp

    HT = HIDDEN // P            # 16 hidden k-tiles
    NT = seq // TB              # token blocks (4)
    ST = seq // P               # 128-token k-tiles (16)
    NW = ST // 2                # wide (1024-col) score tiles per (h, sq) = 8
    scale = 1.0 / float(np.sqrt(HEAD_DIM))

    nc = bacc.Bacc("TRN2", target_bir_lowering=False, debug=False, num_devices=N_CORES)

    # x2 host layout: [p, i, s] = x.T[128i+p, s], so four i-tiles of one
    # token block are a single 3D-AP DMA (16 triggers total instead of 64
    # -- each dma_start costs ~644ns of engine issue time, and trigger
    # serialization was delaying later blocks' transfers by ~5-10us).
    x2_d = nc.dram_tensor("x2", [P, HT, seq], BF16, kind="ExternalInput")
    # wq/wk live d-major on the host: row 128d+p, col 128i+c holds
    # wT[128i+p, 128d+c], so the d-th head's weights are one contiguous
    # [128, HIDDEN] DMA and k-proj head d only waits on 512KB, not 2MB.
    wqT_d = nc.dram_tensor("wqT", [DPC, HIDDEN], BF16, kind="ExternalInput")
    wkT_d = nc.dram_tensor("wkT", [DPC, HIDDEN], BF16, kind="ExternalInput")
    # wv in the same [p, i, o] layout as x2: 4 i-tiles per DMA trigger
    wv2_d = nc.dram_tensor("wv2", [P, HT, DPC], BF16, kind="ExternalInput")
    woT_d = nc.dram_tensor("woT", [DPC, HIDDEN], BF16, kind="ExternalInput")
    cosT_d = nc.dram_tensor("cosT", [HEAD_DIM, seq], BF16, kind="ExternalInput")
    sinT_d = nc.dram_tensor("sinT", [HEAD_DIM, seq], BF16, kind="ExternalInput")
    yT_d = nc.dram_tensor("yT", [HIDDEN, seq], BF16, kind="ExternalOutput")

    with tile.TileContext(nc) as tc, nc.allow_low_precision(
        reason="bf16/fp8 matmul operands; fp32 PSUM accumulation; tol is 2e-2"
    ), (
        tc.tile_pool(name="ones", bufs=1)
    ) as opool, tc.tile_pool(name="const", bufs=1) as cpool, (
        tc.tile_pool(name="kres", bufs=1)
    ) as kpool, tc.tile_pool(name="vres", bufs=1) as vpool, (
        tc.tile_pool(name="wq", bufs=1)
    ) as wqpool, tc.tile_pool(name="wo", bufs=1) as wopool, (
        tc.tile_pool(name="xsh", bufs=1)
    ) as xpool, tc.tile_pool(name="ropetmp", bufs=2) as rope_pool:
        ones_c = opool.tile([P, 1], BF16, tag="ones_c")   # denominator lhsT
        ones_r = opool.tile([1, P], F32R, tag="ones_r")   # replicate lhsT
        ones_f = opool.tile([P, P], F32, tag="ones_f")    # memset staging
        nc.vector.memset(ones_f[:], 1.0)
        nc.vector.tensor_copy(ones_c[:], ones_f[:, 0:1])
        nc.vector.tensor_copy(ones_r[:], ones_f[0:1, :])
        bias_sb = opool.tile([P, 1], F32, tag="bias_sb")
        nc.vector.memset(bias_sb[:], EXP_BIAS)

        cos_sb = cpool.tile([P, seq], BF16, tag="cos")
        sin_sb = cpool.tile([P, seq], BF16, tag="sin")
        # sq=0's q-projection output: computed at the end of phase 1 (before
        # vproj(0)) so its RoPE is long done when attn(0) starts.
        qT0 = [cpool.tile([P, TB], BF16, tag=f"qz{d}", name=f"qz{d}") for d in range(HPC)]
        kT = [kpool.tile([P, seq], BF16, tag=f"kT{h}", name=f"kT{h}") for h in range(HPC)]
        # v resident as one [128, kt, dims] tile so DoubleRow can take
        # [:, 2w:2w+2, head] as a [128, 2, 128] lhsT view.
        v_sb = vpool.tile([P, ST, DPC], EX_DT, tag="v", name="v")
        # d-major: wq_sb[d][:, 128i:128(i+1)] is head-d's k-tile-i lhsT
        wq_sb = [wqpool.tile([P, HIDDEN], BF16, tag=f"wq{d}", name=f"wq{d}") for d in range(HPC)]
        wo_sb = [wopool.tile([P, HIDDEN], BF16, tag=f"wo{i}", name=f"wo{i}") for i in range(HPC)]

        # x stays fully resident in SBUF (64KB/partition), loaded once in
        # phase 1 as [128, 4, 512] chunks (4 i-tiles per DMA). Phase 2
        # then needs NO x DMA at all — q-projections read the residents.
        # block 1 (the head-critical first load) in 8 finer chunks of 2
        # i-tiles for progressive overlap; later blocks in 4-tile chunks
        XCH = {tb: (2 if tb == 1 else 4) for tb in range(NT)}
        x_sb = {
            tb: [
                xpool.tile([P, XCH[tb], TB], BF16, tag=f"x{tb}_{j}", name=f"x{tb}_{j}")
                for j in range(HT // XCH[tb])
            ]
            for tb in range(NT)
        }

        def xap(tb, i, lo=0, hi=TB):
            c = XCH[tb]
            return x_sb[tb][i // c][:, i % c, lo:hi]

        def rope(ps, dst, ts):
            """dst = ps*cos + rot(ps)*sin_signed  (sign folded into sinT)."""
            h2 = P // 2
            tmp = rope_pool.tile([P, TB], BF16, tag="rt", name="rt")
            nc.vector.tensor_mul(tmp[0:h2, :], ps[h2:P, :], sin_sb[0:h2, ts])
            nc.vector.tensor_mul(tmp[h2:P, :], ps[0:h2, :], sin_sb[h2:P, ts])
            nc.vector.tensor_mul(dst, ps[:], cos_sb[:, ts])
            nc.vector.tensor_add(dst, dst, tmp[:])

        # ---- Phase 1: k & v projections (shared x load) ------------------
        # tb order [1,2,3,0] so x(tb=0) stays resident for phase 2's sq=0.
        TB_ORDER = [1, 2, 3, 0]
        x_resident = None
        with (
            tc.tile_pool(name="wkv", bufs=1) as wkvpool,
            tc.tile_pool(name="warm", bufs=1) as warmpool,
            tc.tile_pool(name="ps1", bufs=4, space="PSUM") as ps1,
            tc.tile_pool(name="pswarm", bufs=1, space="PSUM") as pswarm,
        ):
            wk_sb = [wkvpool.tile([P, HIDDEN], BF16, tag=f"wk{d}", name=f"wk{d}") for d in range(HPC)]
            wv_sb = [
                wkvpool.tile([P, 4, DPC], BF16, tag=f"wv{j}", name=f"wv{j}")
                for j in range(HT // 4)
            ]

            # HAM warm-up: ~4.3us of dummy matmuls on memset data while the
            # first x/w DMAs are in flight, so the real stream starts at
            # 2.4GHz instead of 1.2 (the throttle needs ~3.4us of PE busy).
            warm = warmpool.tile([P, TB], BF16, tag="warm", name="warm")
            nc.vector.memset(warm[:], 0.0)
            wps = pswarm.tile([P, TB], F32, tag="wps", name="wps")
            for r in range(7):
                nc.tensor.matmul(wps[:], warm[:, 0:P], warm[:], start=(r == 0), stop=(r == 6))

            # Interleave k-proj and v-proj across blocks so the head of the
            # kernel only needs x(tb1)+wk+cos/sin (~2.75MB) instead of
            # ~6MB: v-proj of a block runs two slots after its k-proj, by
            # which time wv has streamed in.
            def start_x(tb, par):
                ts_ = slice(TB * tb, TB * (tb + 1))
                c = XCH[tb]
                for j in range(HT // c):
                    eng = nc.gpsimd if (j + par) % 2 == 0 else nc.scalar
                    eng.dma_start(
                        out=x_sb[tb][j][:], in_=x2_d[:, c * j : c * (j + 1), ts_]
                    )

            start_x(1, 0)
            # sync-queue weight/const loads, most-urgent first
            nc.sync.dma_start(out=wk_sb[0][:], in_=wkT_d[0:P, :])
            nc.sync.dma_start(out=cos_sb[:, TB : 2 * TB], in_=cosT_d[:, TB : 2 * TB])
            nc.sync.dma_start(out=sin_sb[:, TB : 2 * TB], in_=sinT_d[:, TB : 2 * TB])
            for d in range(1, HPC):
                nc.sync.dma_start(out=wk_sb[d][:], in_=wkT_d[P * d : P * (d + 1), :])
            start_x(2, 1)
            for t2 in [2, 3, 0]:
                t2s = slice(TB * t2, TB * (t2 + 1))
                nc.sync.dma_start(out=cos_sb[:, t2s], in_=cosT_d[:, t2s])
                nc.sync.dma_start(out=sin_sb[:, t2s], in_=sinT_d[:, t2s])
            for j in range(HT // 4):
                nc.sync.dma_start(out=wv_sb[j][:], in_=wv2_d[:, 4 * j : 4 * (j + 1), :])
            start_x(3, 0)
            start_x(0, 1)
            for d in range(HPC):
                nc.sync.dma_start(out=wq_sb[d][:], in_=wqT_d[P * d : P * (d + 1), :])
            for i in range(HPC):
                nc.sync.dma_start(out=wo_sb[i][:], in_=woT_d[P * i : P * (i + 1), :])

            def kproj(tb):
                ts_ = slice(TB * tb, TB * (tb + 1))
                for d in range(HPC):          # k projection + RoPE
                    ps = ps1.tile([P, TB], F32, tag="ps1", name="ps1")
                    for i in range(HT):
                        nc.tensor.matmul(
                            ps[:],
                            wk_sb[d][:, P * i : P * (i + 1)],
                            xap(tb, i),
                            start=(i == 0),
                            stop=(i == HT - 1),
                        )
                    rope(ps, kT[d][:, ts_], ts_)

            def vproj(tb):
                for t in range(TB // P):      # v projection (tokens on psum parts)
                    ps = ps1.tile([P, DPC], F32, tag="ps1", name="ps1")
                    for i in range(HT):
                        nc.tensor.matmul(
                            ps[:],
                            xap(tb, i, P * t, P * (t + 1)),
                            wv_sb[i // 4][:, i % 4, :],
                            start=(i == 0),
                            stop=(i == HT - 1),
                        )
                    # last block's copies go on DVE so attn(0)'s exp stream
                    # isn't queued behind them in the scalar FIFO; earlier
                    # blocks stay on scalar (DVE is rope-bound mid-phase-1)
                    if tb == 0:
                        nc.vector.tensor_copy(v_sb[:, tb * (TB // P) + t, :], ps[:])
                    else:
                        nc.scalar.copy(v_sb[:, tb * (TB // P) + t, :], ps[:])

            kproj(1)
            kproj(2)
            vproj(1)
            kproj(3)
            vproj(2)
            vproj(3)
            kproj(0)
            # qproj(0) before vproj(0): vproj's 64 matmuls cover the RoPE
            # latency so attn(0) never waits on qT.
            for d in range(HPC):
                ps = ps1.tile([P, TB], F32, tag="ps1", name="ps1")
                for i in range(HT):
                    nc.tensor.matmul(
                        ps[:],
                        wq_sb[d][:, P * i : P * (i + 1)],
                        xap(0, i),
                        start=(i == 0),
                        stop=(i == HT - 1),
                    )
                rope(ps, qT0[d][:], slice(0, TB))
            vproj(0)

        # ---- Phase 2: per q-block: qproj -> scores/exp/AV -> out-proj ----
        with (
            tc.tile_pool(name="qblk", bufs=2) as qpool,
            # e0..e5 are produced and consumed within one head (av trails
            # exp by <=2 wides), so one buffer each; e6/e7 are read by the
            # deferred tail in the NEXT head's stream -> double-buffered.
            tc.tile_pool(name="expp", bufs=1) as epool,
            tc.tile_pool(name="expt", bufs=2) as epool_t,
            tc.tile_pool(name="esum", bufs=2) as spool,
            tc.tile_pool(name="avres", bufs=2) as avpool,
            tc.tile_pool(name="recp", bufs=2) as recpool,
            tc.tile_pool(name="ybuf", bufs=4) as ypool,
            # pool open order fixes PSUM bank layout: psav->0-1, psQD->2-3,
            # pss->4-7. Banks 4-7 are untouched by phase 1's ps1 pool
            # (banks 0-3), so attn(0)'s first score matmuls never WAR-wait
            # on phase-1 psum drains.
            tc.tile_pool(name="psav", bufs=2, space="PSUM") as psav,
            tc.tile_pool(name="psQD", bufs=2, space="PSUM") as psQD,
            tc.tile_pool(name="pss", bufs=2, space="PSUM") as pss,
        ):
            def denom_start(es):
                """ones-matmul (denominator row) + 1-row fast reciprocal.
                The ones output borrows row 0 of a qproj-ring PSUM tile; the
                ring partner (next qproj/outproj group) only needs the slot
                back after the reciprocal, 0.7us later."""
                psX = psQD.tile([P, TB], F32, tag="psQD", name="psQD")
                nc.tensor.matmul(psX[0:1, :], ones_c[:], es[:], start=True, stop=True)
                rr1 = recpool.tile([1, TB], F32, tag="rr1", name="rr1")
                nc.vector.reciprocal_approx_fast(out=rr1[:], in_=psX[0:1, :])
                return rr1

            def denom_finish(rr1, ps_av, av):
                """broadcast the reciprocal row on gpsimd, scale av on DVE.
                No PE instruction at all, so the PE queue never blocks."""
                rs = recpool.tile([P, TB], F32, tag="rs", name="rs")
                nc.gpsimd.partition_broadcast(rs[:], rr1[:])
                nc.vector.tensor_mul(av[:], ps_av[:], rs[:])

            def qproj_d(sq, d, qT):
                ts = slice(TB * sq, TB * (sq + 1))
                ps = psQD.tile([P, TB], F32, tag="psQD", name="psQD")
                for i in range(HT):
                    nc.tensor.matmul(
                        ps[:],
                        wq_sb[d][:, P * i : P * (i + 1)],
                        xap(sq, i),
                        start=(i == 0),
                        stop=(i == HT - 1),
                    )
                rope(ps, qT[d][:], ts)

            def qproj(sq):
                qT = [qpool.tile([P, TB], BF16, tag=f"qT{d}", name=f"qT{d}") for d in range(HPC)]
                for d in range(HPC):
                    qproj_d(sq, d, qT)
                return qT

            def yout(ps, do, ts_p, final=False):
                yt = ypool.tile([P, TB], BF16, tag="yt", name="yt")
                # copies on DVE (scalar must stay pure-exp: its FIFO feeds
                # the AV stream); bf16 halves copy + DMA cost. y rides the
                # sync queue (keeping the gpsimd ring idle in the tail
                # avoids its slow end-of-kernel drain); the final block
                # also uses the now-idle scalar queue to halve the drain.
                nc.vector.tensor_copy(yt[:], ps[:])
                eng = nc.scalar if (final and do % 2 == 1) else nc.sync
                eng.dma_start(out=yT_d[P * do : P * (do + 1), ts_p], in_=yt[:])

            def do_full(do, av_p, ts_p):
                ps = psQD.tile([P, TB], F32, tag="psQD", name="psQD")
                for i in range(HPC):
                    nc.tensor.matmul(
                        ps[:],
                        wo_sb[i][:, P * do : P * (do + 1)],
                        av_p[i][:],
                        start=(i == 0),
                        stop=(i == HPC - 1),
                    )
                yout(ps, do, ts_p)

            def attn(sq, qT, has_next, pend_prev):
                # After each head, filler PE work lands so the PE always
                # outruns the exp stream: the next block's q-projection
                # group and a slice of the PREVIOUS block's out-projection
                # (dissolved here so scalar/exp never sees an idle window
                # and the PE never waits on exp at block boundaries).
                av = [avpool.tile([P, TB], BF16, tag=f"av{h}", name=f"av{h}") for h in range(HPC)]
                qT_next = (
                    [qpool.tile([P, TB], BF16, tag=f"qT{d}", name=f"qT{d}") for d in range(HPC)]
                    if has_next else None
                )
                prev = None
                for h in range(HPC):
                    ex = [
                        (epool_t if w >= NW - 2 else epool).tile(
                            [P, 2, TB], EX_DT, tag=f"e{w}", name=f"e{w}"
                        )
                        for w in range(NW)
                    ]
                    ps_av = psav.tile([P, TB], F32, tag="psav", name="psav")
                    tA = spool.tile([P, 2, TB], BF16, tag="tA", name="tA")
                    tB = spool.tile([P, 2, TB], BF16, tag="tB", name="tB")
                    es = spool.tile([P, TB], BF16, tag="es", name="es")

                    def s_mm(w):
                        ps_s = pss.tile([P, 2, TB], F32, tag="pss", name="pss")
                        for half in range(2):
                            kt = 2 * w + half
                            nc.tensor.matmul(
                                ps_s[:, half, :],
                                kT[h][:, P * kt : P * (kt + 1)],
                                qT[h][:],
                                start=True,
                                stop=True,
                            )
                        nc.scalar.activation(ex[w][:], ps_s[:], Exp, scale=scale, bias=bias_sb[:])

                    # NB: default args pin this head's tiles — av_mm is
                    # called from the NEXT head's stream (deferred tail)
                    if FP8_AV:
                        def av_mm(w, ps_av=ps_av, ex=ex, h=h):
                            nc.tensor.matmul(
                                ps_av[:],
                                v_sb[:, 2 * w : 2 * w + 2, P * h : P * (h + 1)],
                                ex[w][:],
                                start=(w == 0),
                                stop=(w == NW - 1),
                                perf_mode=DR,
                            )
                    else:
                        def av_mm(w, ps_av=ps_av, ex=ex, h=h):
                            for half in range(2):
                                kt = 2 * w + half
                                nc.tensor.matmul(
                                    ps_av[:],
                                    v_sb[:, kt, P * h : P * (h + 1)],
                                    ex[w][:, half, :],
                                    start=(kt == 0),
                                    stop=(kt == ST - 1),
                                )

                    # software-pipelined: av trails exp by >=2 wides; the
                    # last two AV matmuls of head h-1 and its denominator all
                    # land inside head h's stream so the exp tail never
                    # stalls the PE.
                    s_mm(0)
                    s_mm(1)
                    av_mm(0)
                    s_mm(2)
                    nc.vector.tensor_add(tA[:], ex[0][:], ex[1][:])
                    s_mm(3)
                    if prev is not None:
                        prev[3](NW - 2)
                        prev[3](NW - 1)
                        rr1_p = denom_start(prev[1])
                    nc.vector.tensor_add(tB[:], ex[2][:], ex[3][:])
                    nc.vector.tensor_add(tA[:], tA[:], tB[:])
                    av_mm(1)
                    s_mm(4)
                    if prev is not None:
                        denom_finish(rr1_p, prev[0], prev[2])
                    av_mm(2)
                    s_mm(5)
                    nc.vector.tensor_add(tB[:], ex[4][:], ex[5][:])
                    nc.vector.tensor_add(tA[:], tA[:], tB[:])
                    av_mm(3)
                    s_mm(6)
                    av_mm(4)
                    s_mm(7)
                    nc.vector.tensor_add(tB[:], ex[6][:], ex[7][:])
                    nc.vector.tensor_add(tA[:], tA[:], tB[:])
                    nc.vector.tensor_add(es[:], tA[:, 0, :], tA[:, 1, :])
                    av_mm(5)
                    prev = (ps_av, es, av[h][:], av_mm)
                    # ---- fillers ----
                    if pend_prev is not None and h == 0:
                        # previous block's head-3 tail: last two AV matmuls
                        # + denominator. The scale of av_p[3] rides under
                        # the following qproj / next-head stream.
                        _, av_p, tail_p = pend_prev
                        tail_p[3](NW - 2)
                        tail_p[3](NW - 1)
                        rr1_t = denom_start(tail_p[1])
                        denom_finish(rr1_t, tail_p[0], tail_p[2])
                    if has_next:
                        qproj_d(sq + 1, h, qT_next)
                    if pend_prev is not None:
                        sq_p, av_p, tail_p = pend_prev
                        ts_p = slice(TB * sq_p, TB * (sq_p + 1))
                        # out-proj chains spread across heads; head 0 takes
                        # one only when qproj covers the av_p[3] latency
                        chains = (
                            ([0], range(1, 6), range(6, 11), range(11, 16))
                            if has_next
                            else ([], range(0, 5), range(5, 10), range(10, 16))
                        )[h]
                        for do in chains:
                            do_full(do, av_p, ts_p)
                return av, prev, qT_next

            def outproj(sq, av, tail):
                # the previous attn's head-3 tail (last two AV matmuls +
                # denominator) runs under the first do-groups; every do-chain
                # consumes av[3] last, so only do0 waits ~1us on the scale.
                # The two open chains are issued BEFORE the tail AV matmuls:
                # they give head-3's last two exps ~1.3us of extra PE cover.
                ts = slice(TB * sq, TB * (sq + 1))
                NCOV = 2
                open_ps = []
                for do in range(NCOV):
                    pool = psav if do % 2 == 0 else psQD
                    tag = "psav" if do % 2 == 0 else "psQD"
                    ps = pool.tile([P, TB], F32, tag=tag, name=tag)
                    for i in range(HPC - 1):
                        nc.tensor.matmul(
                            ps[:],
                            wo_sb[i][:, P * do : P * (do + 1)],
                            av[i][:],
                            start=(i == 0),
                            stop=False,
                        )
                    open_ps.append(ps)
                tail[3](NW - 2)
                tail[3](NW - 1)
                rr1_t = denom_start(tail[1])
                denom_finish(rr1_t, tail[0], tail[2])
                for do in range(HIDDEN // P):
                    if do < NCOV:
                        ps = open_ps[do]
                        nc.tensor.matmul(
                            ps[:],
                            wo_sb[HPC - 1][:, P * do : P * (do + 1)],
                            av[HPC - 1][:],
                            start=False,
                            stop=True,
                        )
                    else:
                        ps = psQD.tile([P, TB], F32, tag="psQD", name="psQD")
                        for i in range(HPC):
                            nc.tensor.matmul(
                                ps[:],
                                wo_sb[i][:, P * do : P * (do + 1)],
                                av[i][:],
                                start=(i == 0),
                                stop=(i == HPC - 1),
                            )
                    yout(ps, do, ts, final=True)

            pend = None      # deferred out-projection (+ head-3 tail)
            qT_cur = qT0
            for sq in range(NT):
                av, pending_d, qT_cur = attn(sq, qT_cur, sq + 1 < NT, pend)
                pend = (sq, av, pending_d)
            outproj(*pend)

    nc.compile()
    return nc


def make_in_maps(hidden_states, wq, wk, wv, wo, seq=SEQ):
    """Host-side sharding: per-core input dict."""
    import ml_dtypes

    bf16 = ml_dtypes.bfloat16
    hs = np.asarray(hidden_states, dtype=np.float32)
    inv_freq = 1.0 / (ROPE_BASE ** (np.arange(0, HEAD_DIM, 2, dtype=np.float32) / HEAD_DIM))
    t = np.arange(seq, dtype=np.float32)
    freqs = np.outer(t, inv_freq)                       # [S, 64]
    emb = np.concatenate([freqs, freqs], axis=-1)       # [S, 128]
    cosT = np.ascontiguousarray(np.cos(emb).T.astype(bf16))        # [128, S]
    sinT = np.sin(emb).T.astype(np.float32)             # [128, S]
    sinT_signed = sinT.copy()
    sinT_signed[: HEAD_DIM // 2, :] *= -1.0             # rot sign folded in
    sinT_signed = np.ascontiguousarray(sinT_signed.astype(bf16))

    def dmajor(wT):
        # B[128d+p, 128i+c] = wT[128i+p, 128d+c]; head-d slice contiguous
        return np.ascontiguousarray(
            wT.reshape(HIDDEN // P, P, HPC, P)
            .transpose(2, 1, 0, 3)
            .reshape(DPC, HIDDEN)
            .astype(bf16)
        )

    # x2[p, i, s] = x.T[128i+p, s]: i-tiles adjacent per partition row so
    # four of them load as one 3D-AP DMA
    xT = [
        np.ascontiguousarray(
            hs[b].T.astype(bf16).reshape(HIDDEN // P, P, seq).transpose(1, 0, 2)
        )
        for b in range(BATCH)
    ]
    in_maps = []
    for c in range(N_CORES):
        b = c // TP
        g = c % TP
        rows = slice(DPC * g, DPC * (g + 1))
        in_maps.append(
            {
                "x2": xT[b],
                "wqT": dmajor(wq[rows, :].T.astype(np.float32)),
                "wkT": dmajor(wk[rows, :].T.astype(np.float32)),
                "wv2": np.ascontiguousarray(
                    wv[rows, :].T.astype(bf16).reshape(HIDDEN // P, P, DPC).transpose(1, 0, 2)
                ),
                "woT": np.ascontiguousarray(wo[:, rows].T.astype(bf16)),
                "cosT": cosT,
                "sinT": sinT_signed,
            }
        )
    return in_maps


def combine_outputs(results, seq=SEQ):
    """Host-side unshard: sum head-group partials per batch, transpose."""
    y = np.zeros((BATCH, seq, HIDDEN), dtype=np.float32)
    for c in range(N_CORES):
        b = c // TP
        y[b] += results[c]["yT"].T.astype(np.float32)
    return y


_NC_CACHE = {}


def kernel(hidden_states, wq, wk, wv, wo):
    _ensure_axon_hooks()
    from concourse.bass_utils import run_bass_kernel_spmd

    if "nc" not in _NC_CACHE:
        _NC_CACHE["nc"] = build(SEQ)
    nc = _NC_CACHE["nc"]
    in_maps = make_in_maps(hidden_states, wq, wk, wv, wo, SEQ)
    res = run_bass_kernel_spmd(nc, in_maps, core_ids=list(range(N_CORES)))
    return combine_outputs(res.results, SEQ)



# revision 82
# speedup vs baseline: 1.0014x; 1.0014x over previous
"""Multi-head attention (RoPE, non-causal) on 8 Trainium2 cores.

Sharding: DP=2 over batch x TP=4 over heads (4 heads per core).
Each core computes, for its (batch, head-group):
    kT   = RoPE(x @ wk.T).T   [per head: 128 x S, head-dim on partitions]
    v    = x @ wv.T           [S x 512, tokens on partitions]
  then per 512-token q-block:
    qT     = RoPE(x @ wq.T).T            [bf16]
    expT   = exp(scale * k.T q)          [transposed scores, bf16]
    denom  = ones^T (wide-tile add tree of expT on DVE)  [1 PE matmul]
    avT    = (v.T @ expT) * recip(denom)
    yT_partial = woT.T-slice @ avT        [2048 x 512]
Host sums the 4 head-group partials per batch and transposes.

All matmul operands are bf16 (same PE column rate as fp32r, half the
SBUF/DMA traffic); accumulation stays fp32 in PSUM; y/cos/sin are bf16
(fp8 measured 3.7% max-rel error in simulation — over the 2e-2 budget).

Schedule: x is loaded ONCE and stays SBUF-resident (64KB/partition);
phase 1 interleaves k-proj/v-proj across token blocks (k1,k2,v1,k3,v2,
v3,k0,q0,v0) so the kernel head only needs x(tb1)+wk d-major slice
+cos/sin before full-rate matmuls; ~4us of dummy matmuls on memset data
warm the HAM clock gate during the initial DMA wait. wq/wk live d-major
in DRAM (one contiguous 512KB DMA per head). In phase 2 every
out-projection is DISSOLVED into the following attention block (chains
spread over heads, next-block q-projection groups after each head) so
the exp stream never sees an idle window and the PE never waits on exp
at block boundaries; each head's last two AV matmuls + denominator ride
in the next head's stream, the final head-3 tail under the last
out-projection's opened chains. PSUM pool order (psav,psQD,pss) pins
scores to banks 4-7 so attn(0) never WAR-waits on phase-1 psum drains.
The reciprocal row is broadcast on gpsimd; all y copies ride the DVE
(scalar stays pure-exp) and y DMAs the sync queue (an active gpsimd DMA
ring at kernel end costs ~8us of serial drain), with the final block
split sync/scalar.
"""

import sys
import types
import numpy as np

HIDDEN = 2048
NUM_HEADS = 16
HEAD_DIM = 128
ROPE_BASE = 10000.0
BATCH = 2
SEQ = 2048

N_CORES = 8
DP = 2            # batch shards
TP = 4            # head-group shards
HPC = NUM_HEADS // TP          # heads per core = 4
DPC = HPC * HEAD_DIM           # head dims per core = 512
P = 128                        # partitions
TB = 512                       # token block (matmul free dim)

FP8_AV = False                 # fp8 AV numerator costs ~3% error: too much
AVMUL_GPSIMD = False           # scalar-engine staging blocks exp: keep on DVE


def _ensure_axon_hooks():
    """bass_utils' trace path imports antenv.axon_hooks, which the container's
    antenv stub lacks. Provide it, backed by the ctypes NTFF hook."""
    import antenv

    if hasattr(antenv, "axon_hooks"):
        return
    try:
        from trn_agent_boot.trn_boot import _ntff_profile_via_ctypes

        hook = _ntff_profile_via_ctypes("/opt/axon/libaxon_pjrt.so")
    except Exception:
        hook = None
    m = types.ModuleType("antenv.axon_hooks")
    m.get_axon_ntff_profile_hook = lambda: hook
    sys.modules["antenv.axon_hooks"] = m
    antenv.axon_hooks = m


def build(seq=SEQ):
    """Build the per-core Bass program (SPMD: same program, per-core data)."""
    import concourse.tile as tile
    from concourse import bacc, mybir

    F32 = mybir.dt.float32
    F32R = mybir.dt.float32r
    BF16 = mybir.dt.bfloat16
    FP8 = mybir.dt.float8e4
    EX_DT = FP8 if FP8_AV else BF16
    EXP_BIAS = -2.0 if FP8_AV else 0.0
    DR = mybir.MatmulPerfMode.DoubleRow
    Exp = mybir.ActivationFunctionType.Exp

    HT = HIDDEN // P            # 16 hidden k-tiles
    NT = seq // TB              # token blocks (4)
    ST = seq // P               # 128-token k-tiles (16)
    NW = ST // 2                # wide (1024-col) score tiles per (h, sq) = 8
    scale = 1.0 / float(np.sqrt(HEAD_DIM))

    nc = bacc.Bacc("TRN2", target_bir_lowering=False, debug=False, num_devices=N_CORES)

    # x2 host layout: [p, i, s] = x.T[128i+p, s], so four i-tiles of one
    # token block are a single 3D-AP DMA (16 triggers total instead of 64
    # -- each dma_start costs ~644ns of engine issue time, and trigger
    # serialization was delaying later blocks' transfers by ~5-10us).
    x2_d = nc.dram_tensor("x2", [P, HT, seq], BF16, kind="ExternalInput")
    # wq/wk live d-major on the host: row 128d+p, col 128i+c holds
    # wT[128i+p, 128d+c], so the d-th head's weights are one contiguous
    # [128, HIDDEN] DMA and k-proj head d only waits on 512KB, not 2MB.
    wqT_d = nc.dram_tensor("wqT", [DPC, HIDDEN], BF16, kind="ExternalInput")
    wkT_d = nc.dram_tensor("wkT", [DPC, HIDDEN], BF16, kind="ExternalInput")
    # wv in the same [p, i, o] layout as x2: 4 i-tiles per DMA trigger
    wv2_d = nc.dram_tensor("wv2", [P, HT, DPC], BF16, kind="ExternalInput")
    woT_d = nc.dram_tensor("woT", [DPC, HIDDEN], BF16, kind="ExternalInput")
    cosT_d = nc.dram_tensor("cosT", [HEAD_DIM, seq], BF16, kind="ExternalInput")
    sinT_d = nc.dram_tensor("sinT", [HEAD_DIM, seq], BF16, kind="ExternalInput")
    yT_d = nc.dram_tensor("yT", [HIDDEN, seq], BF16, kind="ExternalOutput")

    with tile.TileContext(nc) as tc, nc.allow_low_precision(
        reason="bf16/fp8 matmul operands; fp32 PSUM accumulation; tol is 2e-2"
    ), (
        tc.tile_pool(name="ones", bufs=1)
    ) as opool, tc.tile_pool(name="const", bufs=1) as cpool, (
        tc.tile_pool(name="kres", bufs=1)
    ) as kpool, tc.tile_pool(name="vres", bufs=1) as vpool, (
        tc.tile_pool(name="wq", bufs=1)
    ) as wqpool, tc.tile_pool(name="wo", bufs=1) as wopool, (
        tc.tile_pool(name="xsh", bufs=1)
    ) as xpool, tc.tile_pool(name="ropetmp", bufs=2) as rope_pool:
        ones_c = opool.tile([P, 1], BF16, tag="ones_c")   # denominator lhsT
        ones_r = opool.tile([1, P], F32R, tag="ones_r")   # replicate lhsT
        ones_f = opool.tile([P, P], F32, tag="ones_f")    # memset staging
        nc.vector.memset(ones_f[:], 1.0)
        nc.vector.tensor_copy(ones_c[:], ones_f[:, 0:1])
        nc.vector.tensor_copy(ones_r[:], ones_f[0:1, :])
        bias_sb = opool.tile([P, 1], F32, tag="bias_sb")
        nc.vector.memset(bias_sb[:], EXP_BIAS)

        cos_sb = cpool.tile([P, seq], BF16, tag="cos")
        sin_sb = cpool.tile([P, seq], BF16, tag="sin")
        # sq=0's q-projection output: computed at the end of phase 1 (before
        # vproj(0)) so its RoPE is long done when attn(0) starts.
        qT0 = [cpool.tile([P, TB], BF16, tag=f"qz{d}", name=f"qz{d}") for d in range(HPC)]
        kT = [kpool.tile([P, seq], BF16, tag=f"kT{h}", name=f"kT{h}") for h in range(HPC)]
        # v resident as one [128, kt, dims] tile so DoubleRow can take
        # [:, 2w:2w+2, head] as a [128, 2, 128] lhsT view.
        v_sb = vpool.tile([P, ST, DPC], EX_DT, tag="v", name="v")
        # d-major: wq_sb[d][:, 128i:128(i+1)] is head-d's k-tile-i lhsT
        wq_sb = [wqpool.tile([P, HIDDEN], BF16, tag=f"wq{d}", name=f"wq{d}") for d in range(HPC)]
        wo_sb = [wopool.tile([P, HIDDEN], BF16, tag=f"wo{i}", name=f"wo{i}") for i in range(HPC)]

        # x stays fully resident in SBUF (64KB/partition), loaded once in
        # phase 1 as [128, 4, 512] chunks (4 i-tiles per DMA). Phase 2
        # then needs NO x DMA at all — q-projections read the residents.
        x_sb = {
            tb: [
                xpool.tile([P, 4, TB], BF16, tag=f"x{tb}_{j}", name=f"x{tb}_{j}")
                for j in range(HT // 4)
            ]
            for tb in range(NT)
        }

        def xap(tb, i, lo=0, hi=TB):
            return x_sb[tb][i // 4][:, i % 4, lo:hi]

        def rope(ps, dst, ts):
            """dst = ps*cos + rot(ps)*sin_signed  (sign folded into sinT)."""
            h2 = P // 2
            tmp = rope_pool.tile([P, TB], BF16, tag="rt", name="rt")
            nc.vector.tensor_mul(tmp[0:h2, :], ps[h2:P, :], sin_sb[0:h2, ts])
            nc.vector.tensor_mul(tmp[h2:P, :], ps[0:h2, :], sin_sb[h2:P, ts])
            nc.vector.tensor_mul(dst, ps[:], cos_sb[:, ts])
            nc.vector.tensor_add(dst, dst, tmp[:])

        # ---- Phase 1: k & v projections (shared x load) ------------------
        # tb order [1,2,3,0] so x(tb=0) stays resident for phase 2's sq=0.
        TB_ORDER = [1, 2, 3, 0]
        x_resident = None
        with (
            tc.tile_pool(name="wkv", bufs=1) as wkvpool,
            tc.tile_pool(name="warm", bufs=1) as warmpool,
            tc.tile_pool(name="ps1", bufs=4, space="PSUM") as ps1,
            tc.tile_pool(name="pswarm", bufs=1, space="PSUM") as pswarm,
        ):
            wk_sb = [wkvpool.tile([P, HIDDEN], BF16, tag=f"wk{d}", name=f"wk{d}") for d in range(HPC)]
            wv_sb = [
                wkvpool.tile([P, 4, DPC], BF16, tag=f"wv{j}", name=f"wv{j}")
                for j in range(HT // 4)
            ]

            # HAM warm-up: ~4.3us of dummy matmuls on memset data while the
            # first x/w DMAs are in flight, so the real stream starts at
            # 2.4GHz instead of 1.2 (the throttle needs ~3.4us of PE busy).
            warm = warmpool.tile([P, TB], BF16, tag="warm", name="warm")
            nc.vector.memset(warm[:], 0.0)
            wps = pswarm.tile([P, TB], F32, tag="wps", name="wps")
            for r in range(7):
                nc.tensor.matmul(wps[:], warm[:, 0:P], warm[:], start=(r == 0), stop=(r == 6))

            # Interleave k-proj and v-proj across blocks so the head of the
            # kernel only needs x(tb1)+wk+cos/sin (~2.75MB) instead of
            # ~6MB: v-proj of a block runs two slots after its k-proj, by
            # which time wv has streamed in.
            def start_x(tb, par):
                ts_ = slice(TB * tb, TB * (tb + 1))
                for j in range(HT // 4):
                    eng = nc.gpsimd if (j + par) % 2 == 0 else nc.scalar
                    eng.dma_start(
                        out=x_sb[tb][j][:], in_=x2_d[:, 4 * j : 4 * (j + 1), ts_]
                    )

            start_x(1, 0)
            # sync-queue weight/const loads, most-urgent first
            nc.sync.dma_start(out=wk_sb[0][:], in_=wkT_d[0:P, :])
            nc.sync.dma_start(out=cos_sb[:, TB : 2 * TB], in_=cosT_d[:, TB : 2 * TB])
            nc.sync.dma_start(out=sin_sb[:, TB : 2 * TB], in_=sinT_d[:, TB : 2 * TB])
            for d in range(1, HPC):
                nc.sync.dma_start(out=wk_sb[d][:], in_=wkT_d[P * d : P * (d + 1), :])
            start_x(2, 1)
            for t2 in [2, 3, 0]:
                t2s = slice(TB * t2, TB * (t2 + 1))
                nc.sync.dma_start(out=cos_sb[:, t2s], in_=cosT_d[:, t2s])
                nc.sync.dma_start(out=sin_sb[:, t2s], in_=sinT_d[:, t2s])
            for j in range(HT // 4):
                nc.sync.dma_start(out=wv_sb[j][:], in_=wv2_d[:, 4 * j : 4 * (j + 1), :])
            start_x(3, 0)
            start_x(0, 1)
            for d in range(HPC):
                nc.sync.dma_start(out=wq_sb[d][:], in_=wqT_d[P * d : P * (d + 1), :])
            for i in range(HPC):
                nc.sync.dma_start(out=wo_sb[i][:], in_=woT_d[P * i : P * (i + 1), :])

            def kproj(tb):
                ts_ = slice(TB * tb, TB * (tb + 1))
                for d in range(HPC):          # k projection + RoPE
                    ps = ps1.tile([P, TB], F32, tag="ps1", name="ps1")
                    for i in range(HT):
                        nc.tensor.matmul(
                            ps[:],
                            wk_sb[d][:, P * i : P * (i + 1)],
                            xap(tb, i),
                            start=(i == 0),
                            stop=(i == HT - 1),
                        )
                    rope(ps, kT[d][:, ts_], ts_)

            def vproj(tb):
                for t in range(TB // P):      # v projection (tokens on psum parts)
                    ps = ps1.tile([P, DPC], F32, tag="ps1", name="ps1")
                    for i in range(HT):
                        nc.tensor.matmul(
                            ps[:],
                            xap(tb, i, P * t, P * (t + 1)),
                            wv_sb[i // 4][:, i % 4, :],
                            start=(i == 0),
                            stop=(i == HT - 1),
                        )
                    # last block's copies go on DVE so attn(0)'s exp stream
                    # isn't queued behind them in the scalar FIFO; earlier
                    # blocks stay on scalar (DVE is rope-bound mid-phase-1)
                    if tb == 0:
                        nc.vector.tensor_copy(v_sb[:, tb * (TB // P) + t, :], ps[:])
                    else:
                        nc.scalar.copy(v_sb[:, tb * (TB // P) + t, :], ps[:])

            kproj(1)
            kproj(2)
            vproj(1)
            kproj(3)
            vproj(2)
            vproj(3)
            kproj(0)
            # qproj(0) before vproj(0): vproj's 64 matmuls cover the RoPE
            # latency so attn(0) never waits on qT.
            for d in range(HPC):
                ps = ps1.tile([P, TB], F32, tag="ps1", name="ps1")
                for i in range(HT):
                    nc.tensor.matmul(
                        ps[:],
                        wq_sb[d][:, P * i : P * (i + 1)],
                        xap(0, i),
                        start=(i == 0),
                        stop=(i == HT - 1),
                    )
                rope(ps, qT0[d][:], slice(0, TB))
            vproj(0)

        # ---- Phase 2: per q-block: qproj -> scores/exp/AV -> out-proj ----
        with (
            tc.tile_pool(name="qblk", bufs=2) as qpool,
            # e0..e5 are produced and consumed within one head (av trails
            # exp by <=2 wides), so one buffer each; e6/e7 are read by the
            # deferred tail in the NEXT head's stream -> double-buffered.
            tc.tile_pool(name="expp", bufs=1) as epool,
            tc.tile_pool(name="expt", bufs=2) as epool_t,
            tc.tile_pool(name="esum", bufs=2) as spool,
            tc.tile_pool(name="avres", bufs=2) as avpool,
            tc.tile_pool(name="recp", bufs=2) as recpool,
            tc.tile_pool(name="ybuf", bufs=4) as ypool,
            # pool open order fixes PSUM bank layout: psav->0-1, psQD->2-3,
            # pss->4-7. Banks 4-7 are untouched by phase 1's ps1 pool
            # (banks 0-3), so attn(0)'s first score matmuls never WAR-wait
            # on phase-1 psum drains.
            tc.tile_pool(name="psav", bufs=2, space="PSUM") as psav,
            tc.tile_pool(name="psQD", bufs=2, space="PSUM") as psQD,
            tc.tile_pool(name="pss", bufs=2, space="PSUM") as pss,
        ):
            def denom_start(es):
                """ones-matmul (denominator row) + 1-row fast reciprocal.
                The ones output borrows row 0 of a qproj-ring PSUM tile; the
                ring partner (next qproj/outproj group) only needs the slot
                back after the reciprocal, 0.7us later."""
                psX = psQD.tile([P, TB], F32, tag="psQD", name="psQD")
                nc.tensor.matmul(psX[0:1, :], ones_c[:], es[:], start=True, stop=True)
                rr1 = recpool.tile([1, TB], F32, tag="rr1", name="rr1")
                nc.vector.reciprocal_approx_fast(out=rr1[:], in_=psX[0:1, :])
                return rr1

            def denom_finish(rr1, ps_av, av):
                """broadcast the reciprocal row on gpsimd, scale av on DVE.
                No PE instruction at all, so the PE queue never blocks."""
                rs = recpool.tile([P, TB], F32, tag="rs", name="rs")
                nc.gpsimd.partition_broadcast(rs[:], rr1[:])
                nc.vector.tensor_mul(av[:], ps_av[:], rs[:])

            def qproj_d(sq, d, qT):
                ts = slice(TB * sq, TB * (sq + 1))
                ps = psQD.tile([P, TB], F32, tag="psQD", name="psQD")
                for i in range(HT):
                    nc.tensor.matmul(
                        ps[:],
                        wq_sb[d][:, P * i : P * (i + 1)],
                        xap(sq, i),
                        start=(i == 0),
                        stop=(i == HT - 1),
                    )
                rope(ps, qT[d][:], ts)

            def qproj(sq):
                qT = [qpool.tile([P, TB], BF16, tag=f"qT{d}", name=f"qT{d}") for d in range(HPC)]
                for d in range(HPC):
                    qproj_d(sq, d, qT)
                return qT

            def yout(ps, do, ts_p, final=False):
                yt = ypool.tile([P, TB], BF16, tag="yt", name="yt")
                # copies on DVE (scalar must stay pure-exp: its FIFO feeds
                # the AV stream); bf16 halves copy + DMA cost. y rides the
                # sync queue (keeping the gpsimd ring idle in the tail
                # avoids its slow end-of-kernel drain); the final block
                # also uses the now-idle scalar queue to halve the drain.
                nc.vector.tensor_copy(yt[:], ps[:])
                eng = nc.scalar if (final and do % 2 == 1) else nc.sync
                eng.dma_start(out=yT_d[P * do : P * (do + 1), ts_p], in_=yt[:])

            def do_full(do, av_p, ts_p):
                ps = psQD.tile([P, TB], F32, tag="psQD", name="psQD")
                for i in range(HPC):
                    nc.tensor.matmul(
                        ps[:],
                        wo_sb[i][:, P * do : P * (do + 1)],
                        av_p[i][:],
                        start=(i == 0),
                        stop=(i == HPC - 1),
                    )
                yout(ps, do, ts_p)

            def attn(sq, qT, has_next, pend_prev, pre=None):
                # After each head, filler PE work lands so the PE always
                # outruns the exp stream: the next block's q-projection
                # group and a slice of the PREVIOUS block's out-projection
                # (dissolved here so scalar/exp never sees an idle window
                # and the PE never waits on exp at block boundaries).
                av = [avpool.tile([P, TB], BF16, tag=f"av{h}", name=f"av{h}") for h in range(HPC)]
                qT_next = (
                    [qpool.tile([P, TB], BF16, tag=f"qT{d}", name=f"qT{d}") for d in range(HPC)]
                    if has_next else None
                )
                prev = None
                for h in range(HPC):
                    # head 0 of the LAST block starts with its first two
                    # wides' exps already computed (pre-scores issued at
                    # the end of the previous attn, where scalar has slack)
                    use_pre = h == 0 and pre is not None
                    ex = [
                        pre[w]
                        if (use_pre and w < 2)
                        else (epool_t if w >= NW - 2 else epool).tile(
                            [P, 2, TB], EX_DT, tag=f"e{w}", name=f"e{w}"
                        )
                        for w in range(NW)
                    ]
                    ps_av = psav.tile([P, TB], F32, tag="psav", name="psav")
                    tA = spool.tile([P, 2, TB], BF16, tag="tA", name="tA")
                    tB = spool.tile([P, 2, TB], BF16, tag="tB", name="tB")
                    es = spool.tile([P, TB], BF16, tag="es", name="es")

                    def s_mm(w):
                        ps_s = pss.tile([P, 2, TB], F32, tag="pss", name="pss")
                        for half in range(2):
                            kt = 2 * w + half
                            nc.tensor.matmul(
                                ps_s[:, half, :],
                                kT[h][:, P * kt : P * (kt + 1)],
                                qT[h][:],
                                start=True,
                                stop=True,
                            )
                        nc.scalar.activation(ex[w][:], ps_s[:], Exp, scale=scale, bias=bias_sb[:])

                    # NB: default args pin this head's tiles — av_mm is
                    # called from the NEXT head's stream (deferred tail)
                    if FP8_AV:
                        def av_mm(w, ps_av=ps_av, ex=ex, h=h):
                            nc.tensor.matmul(
                                ps_av[:],
                                v_sb[:, 2 * w : 2 * w + 2, P * h : P * (h + 1)],
                                ex[w][:],
                                start=(w == 0),
                                stop=(w == NW - 1),
                                perf_mode=DR,
                            )
                    else:
                        def av_mm(w, ps_av=ps_av, ex=ex, h=h):
                            for half in range(2):
                                kt = 2 * w + half
                                nc.tensor.matmul(
                                    ps_av[:],
                                    v_sb[:, kt, P * h : P * (h + 1)],
                                    ex[w][:, half, :],
                                    start=(kt == 0),
                                    stop=(kt == ST - 1),
                                )

                    # software-pipelined: av trails exp by >=2 wides; the
                    # last two AV matmuls of head h-1 and its denominator all
                    # land inside head h's stream so the exp tail never
                    # stalls the PE.
                    if not use_pre:
                        s_mm(0)
                        s_mm(1)
                    av_mm(0)
                    s_mm(2)
                    nc.vector.tensor_add(tA[:], ex[0][:], ex[1][:])
                    s_mm(3)
                    if prev is not None:
                        prev[3](NW - 2)
                        prev[3](NW - 1)
                        rr1_p = denom_start(prev[1])
                    nc.vector.tensor_add(tB[:], ex[2][:], ex[3][:])
                    nc.vector.tensor_add(tA[:], tA[:], tB[:])
                    av_mm(1)
                    s_mm(4)
                    if prev is not None:
                        denom_finish(rr1_p, prev[0], prev[2])
                    av_mm(2)
                    s_mm(5)
                    nc.vector.tensor_add(tB[:], ex[4][:], ex[5][:])
                    nc.vector.tensor_add(tA[:], tA[:], tB[:])
                    av_mm(3)
                    s_mm(6)
                    av_mm(4)
                    s_mm(7)
                    nc.vector.tensor_add(tB[:], ex[6][:], ex[7][:])
                    nc.vector.tensor_add(tA[:], tA[:], tB[:])
                    nc.vector.tensor_add(es[:], tA[:, 0, :], tA[:, 1, :])
                    av_mm(5)
                    prev = (ps_av, es, av[h][:], av_mm)
                    # ---- fillers ----
                    if pend_prev is not None and h == 0:
                        # previous block's head-3 tail: last two AV matmuls
                        # + denominator. The scale of av_p[3] rides under
                        # the following qproj / next-head stream.
                        _, av_p, tail_p = pend_prev
                        tail_p[3](NW - 2)
                        tail_p[3](NW - 1)
                        rr1_t = denom_start(tail_p[1])
                        denom_finish(rr1_t, tail_p[0], tail_p[2])
                    if has_next:
                        qproj_d(sq + 1, h, qT_next)
                    if pend_prev is not None:
                        sq_p, av_p, tail_p = pend_prev
                        ts_p = slice(TB * sq_p, TB * (sq_p + 1))
                        # out-proj chains spread across heads; head 0 takes
                        # one only when qproj covers the av_p[3] latency
                        chains = (
                            ([0], range(1, 6), range(6, 11), range(11, 16))
                            if has_next
                            else ([], range(0, 5), range(5, 10), range(10, 16))
                        )[h]
                        for do in chains:
                            do_full(do, av_p, ts_p)
                # pre-scores for the LAST block's head 0: its qT is ready
                # (rope ran in this attn's h0 filler) and scalar has slack
                # here, so the next attn starts with exps 2 wides ahead.
                pre_next = None
                if sq == NT - 2:
                    pre_next = []
                    for w in range(2):
                        ex_p = epool.tile([P, 2, TB], EX_DT, tag=f"e{w}", name=f"e{w}")
                        ps_p = pss.tile([P, 2, TB], F32, tag="pss", name="pss")
                        for half in range(2):
                            kt = 2 * w + half
                            nc.tensor.matmul(
                                ps_p[:, half, :],
                                kT[0][:, P * kt : P * (kt + 1)],
                                qT_next[0][:],
                                start=True,
                                stop=True,
                            )
                        nc.scalar.activation(ex_p[:], ps_p[:], Exp, scale=scale, bias=bias_sb[:])
                        pre_next.append(ex_p)
                return av, prev, qT_next, pre_next

            def outproj(sq, av, tail):
                # the previous attn's head-3 tail (last two AV matmuls +
                # denominator) runs under the first do-groups; every do-chain
                # consumes av[3] last, so only do0 waits ~1us on the scale.
                # The two open chains are issued BEFORE the tail AV matmuls:
                # they give head-3's last two exps ~1.3us of extra PE cover.
                ts = slice(TB * sq, TB * (sq + 1))
                NCOV = 2
                open_ps = []
                for do in range(NCOV):
                    pool = psav if do % 2 == 0 else psQD
                    tag = "psav" if do % 2 == 0 else "psQD"
                    ps = pool.tile([P, TB], F32, tag=tag, name=tag)
                    for i in range(HPC - 1):
                        nc.tensor.matmul(
                            ps[:],
                            wo_sb[i][:, P * do : P * (do + 1)],
                            av[i][:],
                            start=(i == 0),
                            stop=False,
                        )
                    open_ps.append(ps)
                tail[3](NW - 2)
                tail[3](NW - 1)
                rr1_t = denom_start(tail[1])
                denom_finish(rr1_t, tail[0], tail[2])
                for do in range(HIDDEN // P):
                    if do < NCOV:
                        ps = open_ps[do]
                        nc.tensor.matmul(
                            ps[:],
                            wo_sb[HPC - 1][:, P * do : P * (do + 1)],
                            av[HPC - 1][:],
                            start=False,
                            stop=True,
                        )
                    else:
                        ps = psQD.tile([P, TB], F32, tag="psQD", name="psQD")
                        for i in range(HPC):
                            nc.tensor.matmul(
                                ps[:],
                                wo_sb[i][:, P * do : P * (do + 1)],
                                av[i][:],
                                start=(i == 0),
                                stop=(i == HPC - 1),
                            )
                    yout(ps, do, ts, final=True)

            pend = None      # deferred out-projection (+ head-3 tail)
            qT_cur = qT0
            pre = None
            for sq in range(NT):
                av, pending_d, qT_cur, pre = attn(sq, qT_cur, sq + 1 < NT, pend, pre)
                pend = (sq, av, pending_d)
            outproj(*pend)

    nc.compile()
    return nc


def make_in_maps(hidden_states, wq, wk, wv, wo, seq=SEQ):
    """Host-side sharding: per-core input dict."""
    import ml_dtypes

    bf16 = ml_dtypes.bfloat16
    hs = np.asarray(hidden_states, dtype=np.float32)
    inv_freq = 1.0 / (ROPE_BASE ** (np.arange(0, HEAD_DIM, 2, dtype=np.float32) / HEAD_DIM))
    t = np.arange(seq, dtype=np.float32)
    freqs = np.outer(t, inv_freq)                       # [S, 64]
    emb = np.concatenate([freqs, freqs], axis=-1)       # [S, 128]
    cosT = np.ascontiguousarray(np.cos(emb).T.astype(bf16))        # [128, S]
    sinT = np.sin(emb).T.astype(np.float32)             # [128, S]
    sinT_signed = sinT.copy()
    sinT_signed[: HEAD_DIM // 2, :] *= -1.0             # rot sign folded in
    sinT_signed = np.ascontiguousarray(sinT_signed.astype(bf16))

    def dmajor(wT):
        # B[128d+p, 128i+c] = wT[128i+p, 128d+c]; head-d slice contiguous
        return np.ascontiguousarray(
            wT.reshape(HIDDEN // P, P, HPC, P)
            .transpose(2, 1, 0, 3)
            .reshape(DPC, HIDDEN)
            .astype(bf16)
        )

    # x2[p, i, s] = x.T[128i+p, s]: i-tiles adjacent per partition row so
    # four of them load as one 3D-AP DMA
    xT = [
        np.ascontiguousarray(
            hs[b].T.astype(bf16).reshape(HIDDEN // P, P, seq).transpose(1, 0, 2)
        )
        for b in range(BATCH)
    ]
    in_maps = []
    for c in range(N_CORES):
        b = c // TP
        g = c % TP
        rows = slice(DPC * g, DPC * (g + 1))
        in_maps.append(
            {
                "x2": xT[b],
                "wqT": dmajor(wq[rows, :].T.astype(np.float32)),
                "wkT": dmajor(wk[rows, :].T.astype(np.float32)),
                "wv2": np.ascontiguousarray(
                    wv[rows, :].T.astype(bf16).reshape(HIDDEN // P, P, DPC).transpose(1, 0, 2)
                ),
                "woT": np.ascontiguousarray(wo[:, rows].T.astype(bf16)),
                "cosT": cosT,
                "sinT": sinT_signed,
            }
        )
    return in_maps


def combine_outputs(results, seq=SEQ):
    """Host-side unshard: sum head-group partials per batch, transpose."""
    y = np.zeros((BATCH, seq, HIDDEN), dtype=np.float32)
    for c in range(N_CORES):
        b = c // TP
        y[b] += results[c]["yT"].T.astype(np.float32)
    return y


_NC_CACHE = {}


def kernel(hidden_states, wq, wk, wv, wo):
    _ensure_axon_hooks()
    from concourse.bass_utils import run_bass_kernel_spmd

    if "nc" not in _NC_CACHE:
        _NC_CACHE["nc"] = build(SEQ)
    nc = _NC_CACHE["nc"]
    in_maps = make_in_maps(hidden_states, wq, wk, wv, wo, SEQ)
    res = run_bass_kernel_spmd(nc, in_maps, core_ids=list(range(N_CORES)))
    return combine_outputs(res.results, SEQ)

